# revision 3
# baseline (speedup 1.0000x reference)
"""Causal relative multi-head attention (prefill) on 8 Trainium2 NeuronCores.

Reference computation (fp32):
    q = x @ Wq.T + bq ; k = x @ Wk.T + bk ; v = x @ Wv.T + bv      [B,S,D]
    p = pos @ Wp.T + bp                                            [S,D]
    scores = causal((q+p) @ k.T / sqrt(dk)) ; attn = softmax(scores)
    out = (attn @ v) @ Wo.T + bo                                   [B,S,D]
with B=4, S=2048, D=1024, H=16, dk=64.

Sharding: batch x head-group. Core c handles batch b=c//2 and head group
g=c%2 (8 heads = 512 of the 1024 qkv/concat dims). Each core computes its
partial output projection; a pairwise AllReduce over {2b, 2b+1} completes
out_linear. Host only transposes/slices inputs and concatenates outputs.

On-device layouts (T = transposed, dims on partitions, seq on free axis):
    phase 1  q_posT/kT [512, S] and v [S, 512] from xposT (fp32r matmuls,
             p folded into q via PSUM accumulation, biases via ACT).
    phase 2  per head-pair flash-style attention in scoresT layout
             [keys, queries]: row-packed K=64 score matmuls, additive
             triangular mask on diagonal blocks, ACT exp -> bf16,
             col-packed M=64 attn@v matmuls + rowsum-via-ones matmuls,
             reciprocal normalize -> concat_oT bf16.
    phase 3  out = concat_oT.T @ WoT + bias (bias added via DVE with a
             host-replicated [128, D] bias tile), pairwise AllReduce,
             copy to the output tensor.
"""

import numpy as np
import ml_dtypes

import concourse.bacc as bacc
import concourse.mybir as mybir
import concourse.tile as tile
from concourse.bass_utils import run_bass_kernel_spmd

F32R = mybir.dt.float32r
F32 = mybir.dt.float32
BF16 = mybir.dt.bfloat16
AFT = mybir.ActivationFunctionType

B, S, D = 4, 2048, 1024
H, DK = 16, 64
N_CORES = 8
GROUP_DIMS = 512              # qkv dims per head group (8 heads x 64)
SB = 256                      # phase-1 seq block (fp32r needs N>=256)
NSB = S // SB                 # 8
QB = 512                      # phase-2 query block
NQB = S // QB                 # 4
NKT = S // 128                # 16 key tiles
MASK_NEG = -30000.0

_PROG = None


def _build_program():
    nc = bacc.Bacc("TRN2", target_bir_lowering=False, debug=False,
                   num_devices=N_CORES)

    xpos_d = nc.dram_tensor("xpos", [16, 128, S], F32R, kind="ExternalInput")
    wqp_d = nc.dram_tensor("wqp", [16, 128, GROUP_DIMS], F32R, kind="ExternalInput")
    wk_d = nc.dram_tensor("wk", [8, 128, GROUP_DIMS], F32R, kind="ExternalInput")
    wv_d = nc.dram_tensor("wv", [8, 128, GROUP_DIMS], F32R, kind="ExternalInput")
    wo_d = nc.dram_tensor("wo", [4, 128, D], BF16, kind="ExternalInput")
    bqp_d = nc.dram_tensor("bqp", [128, 4], F32, kind="ExternalInput")
    bk_d = nc.dram_tensor("bk", [128, 4], F32, kind="ExternalInput")
    bo_d = nc.dram_tensor("bo_bc", [128, D], F32, kind="ExternalInput")
    tri_d = nc.dram_tensor("tri", [128, 128], F32, kind="ExternalInput")
    ones_d = nc.dram_tensor("ones64", [128, 64], BF16, kind="ExternalInput")
    y_d = nc.dram_tensor("y", [S, D], F32, kind="ExternalOutput")

    with tile.TileContext(nc) as tc:
        with (
            tc.tile_pool(name="wts", bufs=1) as wts,
            tc.tile_pool(name="xin", bufs=2) as xin,
            tc.tile_pool(name="big", bufs=1) as big,
            tc.tile_pool(name="att", bufs=4) as att,
            tc.tile_pool(name="rcp", bufs=2) as rcp,
            tc.tile_pool(name="outp", bufs=4) as outp,
            tc.tile_pool(name="cst", bufs=1) as cst,
            tc.tile_pool(name="ps", bufs=2, space="PSUM") as ps,
            tc.tile_pool(name="ps_s", bufs=4, space="PSUM") as ps_s,
            tc.tile_pool(name="ps_acc", bufs=1, space="PSUM") as ps_acc,
            tc.tile_pool(name="dram", bufs=1, space="DRAM") as dram,
        ):
            # ---- constants + weights (resident) ----
            tri_t = cst.tile([128, 128], F32)
            ones_t = cst.tile([128, 64], BF16)
            bqp_t = cst.tile([128, 4], F32)
            bk_t = cst.tile([128, 4], F32)
            bo_t = cst.tile([128, D], F32)
            nc.sync.dma_start(tri_t[:], tri_d[:])
            nc.sync.dma_start(ones_t[:], ones_d[:])
            nc.sync.dma_start(bqp_t[:], bqp_d[:])
            nc.sync.dma_start(bk_t[:], bk_d[:])
            nc.sync.dma_start(bo_t[:], bo_d[:])

            wqp_t = wts.tile([128, 16, GROUP_DIMS], F32R)
            wk_t = wts.tile([128, 8, GROUP_DIMS], F32R)
            wv_t = wts.tile([128, 8, GROUP_DIMS], F32R)
            wo_t = wts.tile([128, 4, D], BF16)
            for i in range(16):
                nc.sync.dma_start(wqp_t[:, i, :], wqp_d[i])
            for i in range(8):
                nc.sync.dma_start(wk_t[:, i, :], wk_d[i])
                nc.sync.dma_start(wv_t[:, i, :], wv_d[i])
            for i in range(4):
                nc.sync.dma_start(wo_t[:, i, :], wo_d[i])

            qT = big.tile([128, 4, S], BF16)      # (q+p)/sqrt(dk), dims x seq
            kT = big.tile([128, 4, S], BF16)
            v_sb = big.tile([128, NKT, GROUP_DIMS], BF16)  # seq x dims
            coT = big.tile([128, 4, S], BF16)     # normalized attn out, dims x seq

            # ---- phase 1: projections ----
            for sb in range(NSB):
                xp_t = xin.tile([128, 16, SB], F32R, tag="xp")
                for i in range(16):
                    nc.sync.dma_start(xp_t[:, i, :],
                                      xpos_d[i, :, sb * SB:(sb + 1) * SB])
                for dt in range(4):
                    psq = ps.tile([128, SB], F32, tag="ps")
                    for i in range(16):
                        nc.tensor.matmul(psq[:],
                                         wqp_t[:, i, dt * 128:(dt + 1) * 128],
                                         xp_t[:, i, :],
                                         start=(i == 0), stop=(i == 15))
                    nc.scalar.activation(qT[:, dt, sb * SB:(sb + 1) * SB], psq[:],
                                         AFT.Identity, bias=bqp_t[:, dt:dt + 1],
                                         scale=0.125)
                for dt in range(4):
                    psk = ps.tile([128, SB], F32, tag="ps")
                    for i in range(8):
                        nc.tensor.matmul(psk[:],
                                         wk_t[:, i, dt * 128:(dt + 1) * 128],
                                         xp_t[:, i, :],
                                         start=(i == 0), stop=(i == 7))
                    nc.scalar.activation(kT[:, dt, sb * SB:(sb + 1) * SB], psk[:],
                                         AFT.Identity, bias=bk_t[:, dt:dt + 1])
                for st in range(SB // 128):
                    psv = ps.tile([128, GROUP_DIMS], F32, tag="ps")
                    for i in range(8):
                        nc.tensor.matmul(psv[:],
                                         xp_t[:, i, st * 128:(st + 1) * 128],
                                         wv_t[:, i, :],
                                         start=(i == 0), stop=(i == 7))
                    nc.vector.tensor_copy(v_sb[:, sb * 2 + st, :], psv[:])

            # ---- phase 2: attention per head pair ----
            for hp in range(4):
                for qb in range(NQB):
                    ps_o = ps_acc.tile([128, QB], F32, tag="o")
                    ps_rs = ps_acc.tile([128, QB], F32, tag="rs")
                    nkt = 4 * qb + 4
                    for kt in range(nkt):
                        d = kt - 4 * qb
                        n0 = max(0, 128 * d)
                        n1 = QB
                        qs0 = qb * QB + n0
                        qs1 = (qb + 1) * QB
                        psa = ps_s.tile([128, QB], F32, tag="s")
                        psb = ps_s.tile([128, QB], F32, tag="s")
                        nc.tensor.matmul(psa[:, n0:n1],
                                         kT[0:64, hp, kt * 128:(kt + 1) * 128],
                                         qT[0:64, hp, qs0:qs1],
                                         start=True, stop=True,
                                         tile_position=(0, 0))
                        nc.tensor.matmul(psb[:, n0:n1],
                                         kT[64:128, hp, kt * 128:(kt + 1) * 128],
                                         qT[64:128, hp, qs0:qs1],
                                         start=True, stop=True,
                                         tile_position=(64, 0))
                        if d >= 0:
                            nc.vector.tensor_add(psa[:, n0:n0 + 128],
                                                 psa[:, n0:n0 + 128], tri_t[:])
                            nc.vector.tensor_add(psb[:, n0:n0 + 128],
                                                 psb[:, n0:n0 + 128], tri_t[:])
                        ea = att.tile([128, QB], BF16, tag="exp")
                        eb = att.tile([128, QB], BF16, tag="exp")
                        nc.scalar.activation(ea[:, n0:n1], psa[:, n0:n1], AFT.Exp)
                        nc.scalar.activation(eb[:, n0:n1], psb[:, n0:n1], AFT.Exp)
                        first = kt == 0
                        last = kt == nkt - 1
                        nc.tensor.matmul(ps_o[0:64, n0:n1],
                                         v_sb[:, kt, hp * 128:hp * 128 + 64],
                                         ea[:, n0:n1], start=first, stop=last,
                                         tile_position=(0, 0))
                        nc.tensor.matmul(ps_o[64:128, n0:n1],
                                         v_sb[:, kt, hp * 128 + 64:hp * 128 + 128],
                                         eb[:, n0:n1], start=first, stop=last,
                                         tile_position=(0, 64))
                        nc.tensor.matmul(ps_rs[0:64, n0:n1], ones_t[:],
                                         ea[:, n0:n1], start=first, stop=last,
                                         tile_position=(0, 0))
                        nc.tensor.matmul(ps_rs[64:128, n0:n1], ones_t[:],
                                         eb[:, n0:n1], start=first, stop=last,
                                         tile_position=(0, 64))
                    rc = rcp.tile([128, QB], F32, tag="recip")
                    nc.vector.reciprocal(rc[:], ps_rs[:])
                    nc.vector.tensor_mul(coT[:, hp, qb * QB:(qb + 1) * QB],
                                         ps_o[:], rc[:])

            # ---- phase 3: output projection + pairwise all-reduce ----
            cci = dram.tile([S, D], F32)
            cco = dram.tile([S, D], F32)
            for sq in range(NKT):
                for ob in range(2):
                    pso = ps.tile([128, 512], F32, tag="ps")
                    for ck in range(4):
                        nc.tensor.matmul(pso[:],
                                         coT[:, ck, sq * 128:(sq + 1) * 128],
                                         wo_t[:, ck, ob * 512:(ob + 1) * 512],
                                         start=(ck == 0), stop=(ck == 3))
                    ot = outp.tile([128, 512], F32, tag="out")
                    nc.vector.tensor_add(ot[:], pso[:],
                                         bo_t[:, ob * 512:(ob + 1) * 512])
                    nc.sync.dma_start(
                        cci[sq * 128:(sq + 1) * 128, ob * 512:(ob + 1) * 512],
                        ot[:])
            nc.gpsimd.collective_compute(
                "AllReduce",
                mybir.AluOpType.add,
                replica_groups=[[0, 1], [2, 3], [4, 5], [6, 7]],
                ins=[cci.opt()],
                outs=[cco.opt()],
            )
            for sq in range(NKT):
                nc.sync.dma_start(y_d[sq * 128:(sq + 1) * 128, :],
                                  cco[sq * 128:(sq + 1) * 128, :])

    nc.compile()
    return nc


def _get_program():
    global _PROG
    if _PROG is None:
        _PROG = _build_program()
    return _PROG


def kernel(x, pos_emb, Wq, bq, Wk, bk, Wv, bv, Wp, bp, Wo, bo):
    x = np.asarray(x, dtype=np.float32)
    pos_emb = np.asarray(pos_emb, dtype=np.float32)
    Wq, bq = np.asarray(Wq, np.float32), np.asarray(bq, np.float32)
    Wk, bk = np.asarray(Wk, np.float32), np.asarray(bk, np.float32)
    Wv, bv = np.asarray(Wv, np.float32), np.asarray(bv, np.float32)
    Wp, bp = np.asarray(Wp, np.float32), np.asarray(bp, np.float32)
    Wo, bo = np.asarray(Wo, np.float32), np.asarray(bo, np.float32)

    nc = _get_program()

    posT = np.ascontiguousarray(pos_emb.T)                      # [D, S]
    tri = np.where(np.arange(128)[:, None] <= np.arange(128)[None, :],
                   np.float32(0.0), np.float32(MASK_NEG)).astype(np.float32)
    ones64 = np.ones((128, 64), dtype=ml_dtypes.bfloat16)

    in_maps = []
    for c in range(N_CORES):
        b, g = divmod(c, 2)
        sl = slice(g * GROUP_DIMS, (g + 1) * GROUP_DIMS)
        xT = np.ascontiguousarray(x[b].T)                       # [D, S]
        xpos = np.concatenate([xT, posT], axis=0).reshape(16, 128, S)
        wqpT = np.concatenate([Wq[sl].T, Wp[sl].T], axis=0)     # [2D, 512]
        wkT = np.ascontiguousarray(Wk[sl].T)                    # [D, 512]
        wvT = np.ascontiguousarray(Wv[sl].T)
        woT = np.ascontiguousarray(Wo[:, sl].T)                 # [512, D]
        bqp = ((bq[sl] + bp[sl]) * 0.125).reshape(4, 128).T     # [128, 4]
        bk2 = bk[sl].reshape(4, 128).T
        bo_eff = bo * 0.5 + bv[sl] @ woT                        # [D]
        bo_bc = np.broadcast_to(bo_eff, (128, D))
        in_maps.append({
            "xpos": xpos,
            "wqp": wqpT.reshape(16, 128, GROUP_DIMS),
            "wk": wkT.reshape(8, 128, GROUP_DIMS),
            "wv": wvT.reshape(8, 128, GROUP_DIMS),
            "wo": woT.reshape(4, 128, D).astype(ml_dtypes.bfloat16),
            "bqp": np.ascontiguousarray(bqp, dtype=np.float32),
            "bk": np.ascontiguousarray(bk2, dtype=np.float32),
            "bo_bc": np.ascontiguousarray(bo_bc, dtype=np.float32),
            "tri": tri,
            "ones64": ones64,
        })

    global _last_in_maps
    _last_in_maps = in_maps

    res = run_bass_kernel_spmd(nc, in_maps, list(range(N_CORES)))
    out = np.stack([res.results[2 * b]["y"] for b in range(B)], axis=0)
    return out.astype(np.float32)


_last_in_maps = None


# revision 4
# speedup vs baseline: 1.5731x; 1.5731x over previous
"""Causal relative multi-head attention (prefill) on 8 Trainium2 NeuronCores.

Reference computation (fp32):
    q = x @ Wq.T + bq ; k = x @ Wk.T + bk ; v = x @ Wv.T + bv      [B,S,D]
    p = pos @ Wp.T + bp                                            [S,D]
    scores = causal((q+p) @ k.T / sqrt(dk)) ; attn = softmax(scores)
    out = (attn @ v) @ Wo.T + bo                                   [B,S,D]
with B=4, S=2048, D=1024, H=16, dk=64.

Sharding: batch x head-group. Core c handles batch b=c//2 and head group
g=c%2 (8 heads = 512 of the 1024 qkv/concat dims). Each core computes its
partial output projection; a pairwise AllReduce over {2b, 2b+1} completes
out_linear. Host only transposes/slices inputs and concatenates outputs.

On-device layouts (T = transposed, dims on partitions, seq on free axis):
    phase 1  q_posT/kT [512, S] and v [S, 512] from xposT (fp32r matmuls,
             p folded into q via host-concatenated [Wq|Wp]/[x;pos],
             scale+bias via DVE tensor_scalar).
    phase 2  per head-pair flash-style attention in scoresT layout
             [keys, queries]: row-packed K=64 score matmuls, additive
             triangular mask on diagonal blocks, ACT exp -> bf16,
             col-packed M=64 attn@v matmuls + rowsum-via-ones matmuls,
             approx-reciprocal normalize -> concat_oT bf16.
    phase 3  out = concat_oT.T @ WoT + bias (bias via DVE add with a
             host-replicated [128, D] bias tile).

The whole thing is software-pipelined along seq: attention query-block
qb only needs projections for seq <= (qb+1)*512, and the output
projection + pairwise AllReduce + output copy run per 512-row chunk, so
collective latency hides under compute of later blocks.
"""

import numpy as np
import ml_dtypes

import concourse.bacc as bacc
import concourse.mybir as mybir
import concourse.tile as tile
from concourse.bass_utils import run_bass_kernel_spmd

F32R = mybir.dt.float32r
F32 = mybir.dt.float32
BF16 = mybir.dt.bfloat16
AFT = mybir.ActivationFunctionType
ALU = mybir.AluOpType

B, S, D = 4, 2048, 1024
H, DK = 16, 64
N_CORES = 8
GROUP_DIMS = 512              # qkv dims per head group (8 heads x 64)
SB = 256                      # phase-1 seq block (fp32r needs N>=256)
NSB = S // SB                 # 8
QB = 512                      # phase-2 query block / output chunk
NQB = S // QB                 # 4
NKT = S // 128                # 16 key tiles
MASK_NEG = -30000.0

_PROG = None
_last_in_maps = None


def _build_program():
    nc = bacc.Bacc("TRN2", target_bir_lowering=False, debug=False,
                   num_devices=N_CORES)

    xpos_d = nc.dram_tensor("xpos", [16, 128, S], F32R, kind="ExternalInput")
    wqp_d = nc.dram_tensor("wqp", [16, 128, GROUP_DIMS], F32R, kind="ExternalInput")
    wk_d = nc.dram_tensor("wk", [8, 128, GROUP_DIMS], F32R, kind="ExternalInput")
    wv_d = nc.dram_tensor("wv", [8, 128, GROUP_DIMS], F32R, kind="ExternalInput")
    wo_d = nc.dram_tensor("wo", [4, 128, D], BF16, kind="ExternalInput")
    bqp_d = nc.dram_tensor("bqp", [128, 4], F32, kind="ExternalInput")
    bk_d = nc.dram_tensor("bk", [128, 4], F32, kind="ExternalInput")
    bo_d = nc.dram_tensor("bo_bc", [128, D], F32, kind="ExternalInput")
    tri_d = nc.dram_tensor("tri", [128, 128], F32, kind="ExternalInput")
    ones_d = nc.dram_tensor("ones64", [128, 64], BF16, kind="ExternalInput")
    y_d = nc.dram_tensor("y", [S, D], F32, kind="ExternalOutput")

    with tile.TileContext(nc) as tc:
        with (
            tc.tile_pool(name="wts", bufs=1) as wts,
            tc.tile_pool(name="xin", bufs=2) as xin,
            tc.tile_pool(name="big", bufs=1) as big,
            tc.tile_pool(name="att", bufs=4) as att,
            tc.tile_pool(name="rcp", bufs=2) as rcp,
            tc.tile_pool(name="outp", bufs=4) as outp,
            tc.tile_pool(name="cst", bufs=1) as cst,
            tc.tile_pool(name="ps", bufs=2, space="PSUM") as ps,
            tc.tile_pool(name="ps_s", bufs=4, space="PSUM") as ps_s,
            tc.tile_pool(name="ps_acc", bufs=1, space="PSUM") as ps_acc,
            tc.tile_pool(name="dram", bufs=1, space="DRAM") as dram,
        ):
            # ---- weights for phase 1 first: they gate the first matmuls ----
            wqp_t = wts.tile([128, 16, GROUP_DIMS], F32R)
            wk_t = wts.tile([128, 8, GROUP_DIMS], F32R)
            wv_t = wts.tile([128, 8, GROUP_DIMS], F32R)
            for i in range(16):
                nc.sync.dma_start(wqp_t[:, i, :], wqp_d[i])
            for i in range(8):
                nc.sync.dma_start(wk_t[:, i, :], wk_d[i])
            for i in range(8):
                nc.sync.dma_start(wv_t[:, i, :], wv_d[i])
            bqp_t = cst.tile([128, 4], F32)
            bk_t = cst.tile([128, 4], F32)
            nc.sync.dma_start(bqp_t[:], bqp_d[:])
            nc.sync.dma_start(bk_t[:], bk_d[:])

            qT = big.tile([128, 4, S], BF16)      # (q+p)/sqrt(dk), dims x seq
            kT = big.tile([128, 4, S], BF16)
            v_sb = big.tile([128, NKT, GROUP_DIMS], BF16)  # seq x dims
            coT = big.tile([128, 4, S], BF16)     # normalized attn out, dims x seq

            def phase1_block(sb):
                xp_t = xin.tile([128, 16, SB], F32R, tag="xp")
                for i in range(16):
                    nc.sync.dma_start(xp_t[:, i, :],
                                      xpos_d[i, :, sb * SB:(sb + 1) * SB])
                for dt in range(4):
                    psq = ps.tile([128, GROUP_DIMS], F32, tag="ps")
                    for i in range(16):
                        nc.tensor.matmul(psq[:, :SB],
                                         wqp_t[:, i, dt * 128:(dt + 1) * 128],
                                         xp_t[:, i, :],
                                         start=(i == 0), stop=(i == 15))
                    nc.vector.tensor_scalar(
                        qT[:, dt, sb * SB:(sb + 1) * SB], psq[:, :SB],
                        0.125, bqp_t[:, dt:dt + 1],
                        op0=ALU.mult, op1=ALU.add)
                for dt in range(4):
                    psk = ps.tile([128, GROUP_DIMS], F32, tag="ps")
                    for i in range(8):
                        nc.tensor.matmul(psk[:, :SB],
                                         wk_t[:, i, dt * 128:(dt + 1) * 128],
                                         xp_t[:, i, :],
                                         start=(i == 0), stop=(i == 7))
                    nc.vector.tensor_scalar_add(
                        kT[:, dt, sb * SB:(sb + 1) * SB], psk[:, :SB],
                        bk_t[:, dt:dt + 1])
                for st in range(SB // 128):
                    psv = ps.tile([128, GROUP_DIMS], F32, tag="ps")
                    for i in range(8):
                        nc.tensor.matmul(psv[:],
                                         xp_t[:, i, st * 128:(st + 1) * 128],
                                         wv_t[:, i, :],
                                         start=(i == 0), stop=(i == 7))
                    nc.vector.tensor_copy(v_sb[:, sb * 2 + st, :], psv[:])

            phase1_block(0)
            phase1_block(1)

            tri_t = cst.tile([128, 128], F32)
            ones_t = cst.tile([128, 64], BF16)
            nc.sync.dma_start(tri_t[:], tri_d[:])
            nc.sync.dma_start(ones_t[:], ones_d[:])
            wo_t = wts.tile([128, 4, D], BF16)
            bo_t = cst.tile([128, D], F32)
            for i in range(4):
                nc.sync.dma_start(wo_t[:, i, :], wo_d[i])
            nc.sync.dma_start(bo_t[:], bo_d[:])

            cci = dram.tile([S, D], F32)
            cco = dram.tile([S, D], F32)

            for qb in range(NQB):
                # ---- phase 2: attention for this query block, all head pairs
                for hp in range(4):
                    ps_o = ps_acc.tile([128, QB], F32, tag="o")
                    ps_rs = ps_acc.tile([128, QB], F32, tag="rs")
                    nkt = 4 * qb + 4
                    for kt in range(nkt):
                        d = kt - 4 * qb
                        n0 = max(0, 128 * d)
                        n1 = QB
                        qs0 = qb * QB + n0
                        qs1 = (qb + 1) * QB
                        psa = ps_s.tile([128, QB], F32, tag="s")
                        psb = ps_s.tile([128, QB], F32, tag="s")
                        nc.tensor.matmul(psa[:, n0:n1],
                                         kT[0:64, hp, kt * 128:(kt + 1) * 128],
                                         qT[0:64, hp, qs0:qs1],
                                         start=True, stop=True,
                                         tile_position=(0, 0))
                        nc.tensor.matmul(psb[:, n0:n1],
                                         kT[64:128, hp, kt * 128:(kt + 1) * 128],
                                         qT[64:128, hp, qs0:qs1],
                                         start=True, stop=True,
                                         tile_position=(64, 0))
                        if d >= 0:
                            nc.vector.tensor_add(psa[:, n0:n0 + 128],
                                                 psa[:, n0:n0 + 128], tri_t[:])
                            nc.vector.tensor_add(psb[:, n0:n0 + 128],
                                                 psb[:, n0:n0 + 128], tri_t[:])
                        ea = att.tile([128, QB], BF16, tag="exp")
                        eb = att.tile([128, QB], BF16, tag="exp")
                        nc.scalar.activation(ea[:, n0:n1], psa[:, n0:n1], AFT.Exp)
                        nc.scalar.activation(eb[:, n0:n1], psb[:, n0:n1], AFT.Exp)
                        first = kt == 0
                        last = kt == nkt - 1
                        nc.tensor.matmul(ps_o[0:64, n0:n1],
                                         v_sb[:, kt, hp * 128:hp * 128 + 64],
                                         ea[:, n0:n1], start=first, stop=last,
                                         tile_position=(0, 0))
                        nc.tensor.matmul(ps_o[64:128, n0:n1],
                                         v_sb[:, kt, hp * 128 + 64:hp * 128 + 128],
                                         eb[:, n0:n1], start=first, stop=last,
                                         tile_position=(0, 64))
                        nc.tensor.matmul(ps_rs[0:64, n0:n1], ones_t[:],
                                         ea[:, n0:n1], start=first, stop=last,
                                         tile_position=(0, 0))
                        nc.tensor.matmul(ps_rs[64:128, n0:n1], ones_t[:],
                                         eb[:, n0:n1], start=first, stop=last,
                                         tile_position=(0, 64))
                    rc = rcp.tile([128, QB], F32, tag="recip")
                    nc.vector.reciprocal_approx_fast(rc[:], ps_rs[:])
                    nc.vector.tensor_mul(coT[:, hp, qb * QB:(qb + 1) * QB],
                                         ps_o[:], rc[:])

                # ---- phase 1 for the next two seq blocks (pipelined) ----
                if qb < NQB - 1:
                    phase1_block(2 * qb + 2)
                    phase1_block(2 * qb + 3)

                # ---- phase 3 + chunked all-reduce for this 512-row chunk ----
                for st in range(4):
                    sq = 4 * qb + st
                    for ob in range(2):
                        pso = ps.tile([128, GROUP_DIMS], F32, tag="ps")
                        for ck in range(4):
                            nc.tensor.matmul(pso[:],
                                             coT[:, ck, sq * 128:(sq + 1) * 128],
                                             wo_t[:, ck, ob * 512:(ob + 1) * 512],
                                             start=(ck == 0), stop=(ck == 3))
                        ot = outp.tile([128, 512], F32, tag="out")
                        nc.vector.tensor_add(ot[:], pso[:],
                                             bo_t[:, ob * 512:(ob + 1) * 512])
                        nc.sync.dma_start(
                            cci[sq * 128:(sq + 1) * 128,
                                ob * 512:(ob + 1) * 512],
                            ot[:])
                nc.gpsimd.collective_compute(
                    "AllReduce",
                    mybir.AluOpType.add,
                    replica_groups=[[0, 1], [2, 3], [4, 5], [6, 7]],
                    ins=[cci[qb * QB:(qb + 1) * QB, :].opt()],
                    outs=[cco[qb * QB:(qb + 1) * QB, :].opt()],
                )
                for st in range(4):
                    sq = 4 * qb + st
                    nc.sync.dma_start(y_d[sq * 128:(sq + 1) * 128, :],
                                      cco[sq * 128:(sq + 1) * 128, :])

    nc.compile()
    return nc


def _get_program():
    global _PROG
    if _PROG is None:
        _PROG = _build_program()
    return _PROG


def kernel(x, pos_emb, Wq, bq, Wk, bk, Wv, bv, Wp, bp, Wo, bo):
    x = np.asarray(x, dtype=np.float32)
    pos_emb = np.asarray(pos_emb, dtype=np.float32)
    Wq, bq = np.asarray(Wq, np.float32), np.asarray(bq, np.float32)
    Wk, bk = np.asarray(Wk, np.float32), np.asarray(bk, np.float32)
    Wv, bv = np.asarray(Wv, np.float32), np.asarray(bv, np.float32)
    Wp, bp = np.asarray(Wp, np.float32), np.asarray(bp, np.float32)
    Wo, bo = np.asarray(Wo, np.float32), np.asarray(bo, np.float32)

    nc = _get_program()

    posT = np.ascontiguousarray(pos_emb.T)                      # [D, S]
    tri = np.where(np.arange(128)[:, None] <= np.arange(128)[None, :],
                   np.float32(0.0), np.float32(MASK_NEG)).astype(np.float32)
    ones64 = np.ones((128, 64), dtype=ml_dtypes.bfloat16)

    in_maps = []
    for c in range(N_CORES):
        b, g = divmod(c, 2)
        sl = slice(g * GROUP_DIMS, (g + 1) * GROUP_DIMS)
        xT = np.ascontiguousarray(x[b].T)                       # [D, S]
        xpos = np.concatenate([xT, posT], axis=0).reshape(16, 128, S)
        wqpT = np.concatenate([Wq[sl].T, Wp[sl].T], axis=0)     # [2D, 512]
        wkT = np.ascontiguousarray(Wk[sl].T)                    # [D, 512]
        wvT = np.ascontiguousarray(Wv[sl].T)
        woT = np.ascontiguousarray(Wo[:, sl].T)                 # [512, D]
        bqp = ((bq[sl] + bp[sl]) * 0.125).reshape(4, 128).T     # [128, 4]
        bk2 = bk[sl].reshape(4, 128).T
        bo_eff = bo * 0.5 + bv[sl] @ woT                        # [D]
        bo_bc = np.broadcast_to(bo_eff, (128, D))
        in_maps.append({
            "xpos": xpos,
            "wqp": wqpT.reshape(16, 128, GROUP_DIMS),
            "wk": wkT.reshape(8, 128, GROUP_DIMS),
            "wv": wvT.reshape(8, 128, GROUP_DIMS),
            "wo": woT.reshape(4, 128, D).astype(ml_dtypes.bfloat16),
            "bqp": np.ascontiguousarray(bqp, dtype=np.float32),
            "bk": np.ascontiguousarray(bk2, dtype=np.float32),
            "bo_bc": np.ascontiguousarray(bo_bc, dtype=np.float32),
            "tri": tri,
            "ones64": ones64,
        })

    global _last_in_maps
    _last_in_maps = in_maps

    res = run_bass_kernel_spmd(nc, in_maps, list(range(N_CORES)))
    out = np.stack([res.results[2 * b]["y"] for b in range(B)], axis=0)
    return out.astype(np.float32)


# revision 6
# speedup vs baseline: 1.6979x; 1.0793x over previous
"""Causal relative multi-head attention (prefill) on 8 Trainium2 NeuronCores.

Reference computation (fp32):
    q = x @ Wq.T + bq ; k = x @ Wk.T + bk ; v = x @ Wv.T + bv      [B,S,D]
    p = pos @ Wp.T + bp                                            [S,D]
    scores = causal((q+p) @ k.T / sqrt(dk)) ; attn = softmax(scores)
    out = (attn @ v) @ Wo.T + bo                                   [B,S,D]
with B=4, S=2048, D=1024, H=16, dk=64.

Sharding: batch x head-group. Core c handles batch b=c//2 and head group
g=c%2 (8 heads = 512 of the 1024 qkv/concat dims). Each core computes its
partial output projection; a pairwise AllReduce over {2b, 2b+1} completes
out_linear. Host only transposes/slices inputs and concatenates outputs.

On-device layouts (T = transposed, dims on partitions, seq on free axis):
    phase 1  q_posT/kT [512, S] and v [S, 512] from xposT (fp32r matmuls,
             p folded into q via host-concatenated [Wq|Wp]/[x;pos],
             scale+bias via DVE tensor_scalar).
    phase 2  per head-pair flash-style attention in scoresT layout
             [keys, queries]: row-packed K=64 score matmuls, additive
             triangular mask on diagonal blocks, ACT exp -> bf16,
             col-packed M=64 attn@v matmuls + rowsum-via-ones matmuls,
             approx-reciprocal normalize -> concat_oT bf16.
    phase 3  out = concat_oT.T @ WoT + bias (bias via DVE add with a
             host-replicated [128, D] bias tile).

The whole thing is software-pipelined along seq: attention query-block
qb only needs projections for seq <= (qb+1)*512, and the output
projection + pairwise AllReduce + output copy run per 512-row chunk, so
collective latency hides under compute of later blocks.
"""

import numpy as np
import ml_dtypes

import concourse.bacc as bacc
import concourse.mybir as mybir
import concourse.tile as tile
from concourse.bass_utils import run_bass_kernel_spmd

F32R = mybir.dt.float32r
F32 = mybir.dt.float32
BF16 = mybir.dt.bfloat16
AFT = mybir.ActivationFunctionType
ALU = mybir.AluOpType

B, S, D = 4, 2048, 1024
H, DK = 16, 64
N_CORES = 8
GROUP_DIMS = 512              # qkv dims per head group (8 heads x 64)
SB = 512                      # phase-1 seq block
NSB = S // SB                 # 4
QB = 512                      # phase-2 query block / output chunk
NQB = S // QB                 # 4
NKT = S // 128                # 16 key tiles
MASK_NEG = -30000.0

_PROG = None
_last_in_maps = None


def _build_program():
    nc = bacc.Bacc("TRN2", target_bir_lowering=False, debug=False,
                   num_devices=N_CORES)

    xpos_d = nc.dram_tensor("xpos", [16, 128, S], BF16, kind="ExternalInput")
    wqp_d = nc.dram_tensor("wqp", [16, 128, GROUP_DIMS], BF16, kind="ExternalInput")
    wk_d = nc.dram_tensor("wk", [8, 128, GROUP_DIMS], BF16, kind="ExternalInput")
    wv_d = nc.dram_tensor("wv", [8, 128, GROUP_DIMS], BF16, kind="ExternalInput")
    wo_d = nc.dram_tensor("wo", [4, 128, D], BF16, kind="ExternalInput")
    bqp_d = nc.dram_tensor("bqp", [128, 4], F32, kind="ExternalInput")
    bk_d = nc.dram_tensor("bk", [128, 4], F32, kind="ExternalInput")
    bo_d = nc.dram_tensor("bo_bc", [128, D], F32, kind="ExternalInput")
    tri_d = nc.dram_tensor("tri", [128, 128], F32, kind="ExternalInput")
    ones_d = nc.dram_tensor("ones64", [128, 64], BF16, kind="ExternalInput")
    y_d = nc.dram_tensor("y", [S, D], F32, kind="ExternalOutput")

    with tile.TileContext(nc) as tc:
        with (
            tc.tile_pool(name="wts", bufs=1) as wts,
            tc.tile_pool(name="xin", bufs=2) as xin,
            tc.tile_pool(name="big", bufs=1) as big,
            tc.tile_pool(name="att", bufs=4) as att,
            tc.tile_pool(name="rcp", bufs=2) as rcp,
            tc.tile_pool(name="outp", bufs=4) as outp,
            tc.tile_pool(name="cst", bufs=1) as cst,
            tc.tile_pool(name="ps", bufs=2, space="PSUM") as ps,
            tc.tile_pool(name="ps_s", bufs=4, space="PSUM") as ps_s,
            tc.tile_pool(name="ps_acc", bufs=1, space="PSUM") as ps_acc,
            tc.tile_pool(name="dram", bufs=1, space="DRAM") as dram,
        ):
            # ---- weights for phase 1 first: they gate the first matmuls ----
            wqp_t = wts.tile([128, 16, GROUP_DIMS], BF16)
            wk_t = wts.tile([128, 8, GROUP_DIMS], BF16)
            wv_t = wts.tile([128, 8, GROUP_DIMS], BF16)
            for i in range(16):
                nc.sync.dma_start(wqp_t[:, i, :], wqp_d[i])
            for i in range(8):
                nc.sync.dma_start(wk_t[:, i, :], wk_d[i])
            for i in range(8):
                nc.sync.dma_start(wv_t[:, i, :], wv_d[i])
            bqp_t = cst.tile([128, 4], F32)
            bk_t = cst.tile([128, 4], F32)
            nc.sync.dma_start(bqp_t[:], bqp_d[:])
            nc.sync.dma_start(bk_t[:], bk_d[:])

            qT = big.tile([128, 4, S], BF16)      # (q+p)/sqrt(dk), dims x seq
            kT = big.tile([128, 4, S], BF16)
            v_sb = big.tile([128, NKT, GROUP_DIMS], BF16)  # seq x dims
            coT = big.tile([128, 4, S], BF16)     # normalized attn out, dims x seq

            def phase1_block(sb):
                xp_t = xin.tile([128, 16, SB], BF16, tag="xp")
                for i in range(16):
                    nc.sync.dma_start(xp_t[:, i, :],
                                      xpos_d[i, :, sb * SB:(sb + 1) * SB])
                for dt in range(4):
                    psq = ps.tile([128, GROUP_DIMS], F32, tag="ps")
                    for i in range(16):
                        nc.tensor.matmul(psq[:, :SB],
                                         wqp_t[:, i, dt * 128:(dt + 1) * 128],
                                         xp_t[:, i, :],
                                         start=(i == 0), stop=(i == 15))
                    nc.vector.tensor_scalar(
                        qT[:, dt, sb * SB:(sb + 1) * SB], psq[:, :SB],
                        0.125, bqp_t[:, dt:dt + 1],
                        op0=ALU.mult, op1=ALU.add)
                for dt in range(4):
                    psk = ps.tile([128, GROUP_DIMS], F32, tag="ps")
                    for i in range(8):
                        nc.tensor.matmul(psk[:, :SB],
                                         wk_t[:, i, dt * 128:(dt + 1) * 128],
                                         xp_t[:, i, :],
                                         start=(i == 0), stop=(i == 7))
                    nc.vector.tensor_scalar_add(
                        kT[:, dt, sb * SB:(sb + 1) * SB], psk[:, :SB],
                        bk_t[:, dt:dt + 1])
                for st in range(SB // 128):
                    psv = ps.tile([128, GROUP_DIMS], F32, tag="ps")
                    for i in range(8):
                        nc.tensor.matmul(psv[:],
                                         xp_t[:, i, st * 128:(st + 1) * 128],
                                         wv_t[:, i, :],
                                         start=(i == 0), stop=(i == 7))
                    nc.vector.tensor_copy(v_sb[:, sb * 4 + st, :], psv[:])

            phase1_block(0)

            tri_t = cst.tile([128, 128], F32)
            ones_t = cst.tile([128, 64], BF16)
            nc.sync.dma_start(tri_t[:], tri_d[:])
            nc.sync.dma_start(ones_t[:], ones_d[:])
            wo_t = wts.tile([128, 4, D], BF16)
            bo_t = cst.tile([128, D], F32)
            for i in range(4):
                nc.sync.dma_start(wo_t[:, i, :], wo_d[i])
            nc.sync.dma_start(bo_t[:], bo_d[:])

            cci = dram.tile([S, D], F32)
            cco = dram.tile([S, D], F32)

            for qb in range(NQB):
                # ---- phase 2: attention for this query block, all head pairs
                for hp in range(4):
                    ps_o = ps_acc.tile([128, QB], F32, tag="o")
                    ps_rs = ps_acc.tile([128, QB], F32, tag="rs")
                    nkt = 4 * qb + 4
                    for kt in range(nkt):
                        d = kt - 4 * qb
                        n0 = max(0, 128 * d)
                        n1 = QB
                        qs0 = qb * QB + n0
                        qs1 = (qb + 1) * QB
                        psa = ps_s.tile([128, QB], F32, tag="s")
                        psb = ps_s.tile([128, QB], F32, tag="s")
                        nc.tensor.matmul(psa[:, n0:n1],
                                         kT[0:64, hp, kt * 128:(kt + 1) * 128],
                                         qT[0:64, hp, qs0:qs1],
                                         start=True, stop=True,
                                         tile_position=(0, 0))
                        nc.tensor.matmul(psb[:, n0:n1],
                                         kT[64:128, hp, kt * 128:(kt + 1) * 128],
                                         qT[64:128, hp, qs0:qs1],
                                         start=True, stop=True,
                                         tile_position=(64, 0))
                        if d >= 0:
                            nc.vector.tensor_add(psa[:, n0:n0 + 128],
                                                 psa[:, n0:n0 + 128], tri_t[:])
                            nc.vector.tensor_add(psb[:, n0:n0 + 128],
                                                 psb[:, n0:n0 + 128], tri_t[:])
                        ea = att.tile([128, QB], BF16, tag="exp")
                        eb = att.tile([128, QB], BF16, tag="exp")
                        nc.scalar.activation(ea[:, n0:n1], psa[:, n0:n1], AFT.Exp)
                        nc.scalar.activation(eb[:, n0:n1], psb[:, n0:n1], AFT.Exp)
                        first = kt == 0
                        last = kt == nkt - 1
                        nc.tensor.matmul(ps_o[0:64, n0:n1],
                                         v_sb[:, kt, hp * 128:hp * 128 + 64],
                                         ea[:, n0:n1], start=first, stop=last,
                                         tile_position=(0, 0))
                        nc.tensor.matmul(ps_o[64:128, n0:n1],
                                         v_sb[:, kt, hp * 128 + 64:hp * 128 + 128],
                                         eb[:, n0:n1], start=first, stop=last,
                                         tile_position=(0, 64))
                        nc.tensor.matmul(ps_rs[0:64, n0:n1], ones_t[:],
                                         ea[:, n0:n1], start=first, stop=last,
                                         tile_position=(0, 0))
                        nc.tensor.matmul(ps_rs[64:128, n0:n1], ones_t[:],
                                         eb[:, n0:n1], start=first, stop=last,
                                         tile_position=(0, 64))
                    rc = rcp.tile([128, QB], F32, tag="recip")
                    nc.vector.reciprocal_approx_fast(rc[:], ps_rs[:])
                    nc.vector.tensor_mul(coT[:, hp, qb * QB:(qb + 1) * QB],
                                         ps_o[:], rc[:])

                # ---- phase 1 for the next seq block (pipelined) ----
                if qb < NQB - 1:
                    phase1_block(qb + 1)

                # ---- phase 3 + chunked all-reduce for this 512-row chunk ----
                for st in range(4):
                    sq = 4 * qb + st
                    for ob in range(2):
                        pso = ps.tile([128, GROUP_DIMS], F32, tag="ps")
                        for ck in range(4):
                            nc.tensor.matmul(pso[:],
                                             coT[:, ck, sq * 128:(sq + 1) * 128],
                                             wo_t[:, ck, ob * 512:(ob + 1) * 512],
                                             start=(ck == 0), stop=(ck == 3))
                        ot = outp.tile([128, 512], F32, tag="out")
                        nc.vector.tensor_add(ot[:], pso[:],
                                             bo_t[:, ob * 512:(ob + 1) * 512])
                        nc.sync.dma_start(
                            cci[sq * 128:(sq + 1) * 128,
                                ob * 512:(ob + 1) * 512],
                            ot[:])
                nc.gpsimd.collective_compute(
                    "AllReduce",
                    mybir.AluOpType.add,
                    replica_groups=[[0, 1], [2, 3], [4, 5], [6, 7]],
                    ins=[cci[qb * QB:(qb + 1) * QB, :].opt()],
                    outs=[cco[qb * QB:(qb + 1) * QB, :].opt()],
                )
                for st in range(4):
                    sq = 4 * qb + st
                    nc.sync.dma_start(y_d[sq * 128:(sq + 1) * 128, :],
                                      cco[sq * 128:(sq + 1) * 128, :])

    nc.compile()
    return nc


def _get_program():
    global _PROG
    if _PROG is None:
        _PROG = _build_program()
    return _PROG


def kernel(x, pos_emb, Wq, bq, Wk, bk, Wv, bv, Wp, bp, Wo, bo):
    x = np.asarray(x, dtype=np.float32)
    pos_emb = np.asarray(pos_emb, dtype=np.float32)
    Wq, bq = np.asarray(Wq, np.float32), np.asarray(bq, np.float32)
    Wk, bk = np.asarray(Wk, np.float32), np.asarray(bk, np.float32)
    Wv, bv = np.asarray(Wv, np.float32), np.asarray(bv, np.float32)
    Wp, bp = np.asarray(Wp, np.float32), np.asarray(bp, np.float32)
    Wo, bo = np.asarray(Wo, np.float32), np.asarray(bo, np.float32)

    nc = _get_program()

    posT = np.ascontiguousarray(pos_emb.T)                      # [D, S]
    tri = np.where(np.arange(128)[:, None] <= np.arange(128)[None, :],
                   np.float32(0.0), np.float32(MASK_NEG)).astype(np.float32)
    ones64 = np.ones((128, 64), dtype=ml_dtypes.bfloat16)

    in_maps = []
    for c in range(N_CORES):
        b, g = divmod(c, 2)
        sl = slice(g * GROUP_DIMS, (g + 1) * GROUP_DIMS)
        xT = np.ascontiguousarray(x[b].T)                       # [D, S]
        xpos = np.concatenate([xT, posT], axis=0).reshape(16, 128, S)
        wqpT = np.concatenate([Wq[sl].T, Wp[sl].T], axis=0)     # [2D, 512]
        wkT = np.ascontiguousarray(Wk[sl].T)                    # [D, 512]
        wvT = np.ascontiguousarray(Wv[sl].T)
        woT = np.ascontiguousarray(Wo[:, sl].T)                 # [512, D]
        bqp = ((bq[sl] + bp[sl]) * 0.125).reshape(4, 128).T     # [128, 4]
        bk2 = bk[sl].reshape(4, 128).T
        bo_eff = bo * 0.5 + bv[sl] @ woT                        # [D]
        bo_bc = np.broadcast_to(bo_eff, (128, D))
        in_maps.append({
            "xpos": xpos.astype(ml_dtypes.bfloat16),
            "wqp": wqpT.reshape(16, 128, GROUP_DIMS).astype(ml_dtypes.bfloat16),
            "wk": wkT.reshape(8, 128, GROUP_DIMS).astype(ml_dtypes.bfloat16),
            "wv": wvT.reshape(8, 128, GROUP_DIMS).astype(ml_dtypes.bfloat16),
            "wo": woT.reshape(4, 128, D).astype(ml_dtypes.bfloat16),
            "bqp": np.ascontiguousarray(bqp, dtype=np.float32),
            "bk": np.ascontiguousarray(bk2, dtype=np.float32),
            "bo_bc": np.ascontiguousarray(bo_bc, dtype=np.float32),
            "tri": tri,
            "ones64": ones64,
        })

    global _last_in_maps
    _last_in_maps = in_maps

    res = run_bass_kernel_spmd(nc, in_maps, list(range(N_CORES)))
    out = np.stack([res.results[2 * b]["y"] for b in range(B)], axis=0)
    return out.astype(np.float32)


# revision 11
# speedup vs baseline: 1.8085x; 1.0651x over previous
"""Causal relative multi-head attention (prefill) on 8 Trainium2 NeuronCores.

Reference computation (fp32):
    q = x @ Wq.T + bq ; k = x @ Wk.T + bk ; v = x @ Wv.T + bv      [B,S,D]
    p = pos @ Wp.T + bp                                            [S,D]
    scores = causal((q+p) @ k.T / sqrt(dk)) ; attn = softmax(scores)
    out = (attn @ v) @ Wo.T + bo                                   [B,S,D]
with B=4, S=2048, D=1024, H=16, dk=64.

Sharding: batch x head-group. Core c handles batch b=c//2 and head group
g=c%2 (8 heads = 512 of the 1024 qkv/concat dims). Each core computes its
partial output projection; a pairwise AllReduce over {2b, 2b+1} completes
out_linear. Host only transposes/slices inputs and concatenates outputs.

On-device layouts (T = transposed, dims on partitions, seq on free axis):
    phase 1  q_posT/kT [512, S] and v [S, 512] from xposT (fp32r matmuls,
             p folded into q via host-concatenated [Wq|Wp]/[x;pos],
             scale+bias via DVE tensor_scalar).
    phase 2  per head-pair flash-style attention in scoresT layout
             [keys, queries]: row-packed K=64 score matmuls, additive
             triangular mask on diagonal blocks, ACT exp -> bf16,
             col-packed M=64 attn@v matmuls + rowsum-via-ones matmuls,
             approx-reciprocal normalize -> concat_oT bf16.
    phase 3  out = concat_oT.T @ WoT + bias (bias via DVE add with a
             host-replicated [128, D] bias tile).

The whole thing is software-pipelined along seq: attention query-block
qb only needs projections for seq <= (qb+1)*512, and the output
projection + pairwise AllReduce + output copy run per 512-row chunk, so
collective latency hides under compute of later blocks.
"""

import numpy as np
import ml_dtypes

import concourse.bacc as bacc
import concourse.mybir as mybir
import concourse.tile as tile
from concourse.bass_utils import run_bass_kernel_spmd

F32R = mybir.dt.float32r
F32 = mybir.dt.float32
BF16 = mybir.dt.bfloat16
AFT = mybir.ActivationFunctionType
ALU = mybir.AluOpType

B, S, D = 4, 2048, 1024
H, DK = 16, 64
N_CORES = 8
GROUP_DIMS = 512              # qkv dims per head group (8 heads x 64)
SB = 512                      # phase-1 seq block
NSB = S // SB                 # 4
QB = 512                      # phase-2 query block / output chunk
NQB = S // QB                 # 4
NKT = S // 128                # 16 key tiles
MASK_NEG = -30000.0

_PROG = None
_last_in_maps = None


def _build_program():
    nc = bacc.Bacc("TRN2", target_bir_lowering=False, debug=False,
                   num_devices=N_CORES)

    xpos_d = nc.dram_tensor("xpos", [16, 128, S], BF16, kind="ExternalInput")
    wqp_d = nc.dram_tensor("wqp", [16, 128, GROUP_DIMS], BF16, kind="ExternalInput")
    wk_d = nc.dram_tensor("wk", [8, 128, GROUP_DIMS], BF16, kind="ExternalInput")
    wv_d = nc.dram_tensor("wv", [8, 128, GROUP_DIMS], BF16, kind="ExternalInput")
    wo_d = nc.dram_tensor("wo", [4, 128, D], BF16, kind="ExternalInput")
    bqp_d = nc.dram_tensor("bqp", [128, 4], F32, kind="ExternalInput")
    bk_d = nc.dram_tensor("bk", [128, 4], F32, kind="ExternalInput")
    bo_d = nc.dram_tensor("bo_bc", [128, D], F32, kind="ExternalInput")
    tri_d = nc.dram_tensor("tri", [128, 128], F32, kind="ExternalInput")
    ones_d = nc.dram_tensor("ones64", [128, 64], BF16, kind="ExternalInput")
    y_d = nc.dram_tensor("y", [S, D], F32, kind="ExternalOutput")

    with tile.TileContext(nc) as tc:
        with (
            tc.tile_pool(name="wts", bufs=1) as wts,
            tc.tile_pool(name="xin", bufs=2) as xin,
            tc.tile_pool(name="big", bufs=1) as big,
            tc.tile_pool(name="att", bufs=4) as att,
            tc.tile_pool(name="rcp", bufs=2) as rcp,
            tc.tile_pool(name="outp", bufs=4) as outp,
            tc.tile_pool(name="cst", bufs=1) as cst,
            tc.tile_pool(name="ps", bufs=2, space="PSUM") as ps,
            tc.tile_pool(name="ps_s", bufs=2, space="PSUM") as ps_s,
            tc.tile_pool(name="ps_acc", bufs=1, space="PSUM") as ps_acc,
            tc.tile_pool(name="dram", bufs=1, space="DRAM") as dram,
        ):
            # ---- weights for phase 1 first: they gate the first matmuls ----
            wqp_t = wts.tile([128, 16, GROUP_DIMS], BF16)
            wk_t = wts.tile([128, 8, GROUP_DIMS], BF16)
            wv_t = wts.tile([128, 8, GROUP_DIMS], BF16)
            for i in range(16):
                nc.sync.dma_start(wqp_t[:, i, :], wqp_d[i])
            for i in range(8):
                nc.sync.dma_start(wk_t[:, i, :], wk_d[i])
            for i in range(8):
                nc.sync.dma_start(wv_t[:, i, :], wv_d[i])
            bqp_t = cst.tile([128, 4], F32)
            bk_t = cst.tile([128, 4], F32)
            nc.sync.dma_start(bqp_t[:], bqp_d[:])
            nc.sync.dma_start(bk_t[:], bk_d[:])

            qT = big.tile([128, 4, S], BF16)      # (q+p)/sqrt(dk), dims x seq
            kT = big.tile([128, 4, S], BF16)
            v_sb = big.tile([128, NKT, 4, 256], BF16)  # seq x [vA|1|vB|1] per pair
            coT = big.tile([128, 4, S], BF16)     # normalized attn out, dims x seq
            nc.gpsimd.memset(v_sb[:, :, :, 64:128], 1.0)
            nc.gpsimd.memset(v_sb[:, :, :, 192:256], 1.0)

            def phase1_block(sb):
                xp_t = xin.tile([128, 16, SB], BF16, tag="xp")
                for i in range(16):
                    nc.sync.dma_start(xp_t[:, i, :],
                                      xpos_d[i, :, sb * SB:(sb + 1) * SB])
                for dt in range(4):
                    psq = ps.tile([128, GROUP_DIMS], F32, tag="ps")
                    for i in range(16):
                        nc.tensor.matmul(psq[:, :SB],
                                         wqp_t[:, i, dt * 128:(dt + 1) * 128],
                                         xp_t[:, i, :],
                                         start=(i == 0), stop=(i == 15))
                    nc.vector.tensor_scalar(
                        qT[:, dt, sb * SB:(sb + 1) * SB], psq[:, :SB],
                        0.125, bqp_t[:, dt:dt + 1],
                        op0=ALU.mult, op1=ALU.add)
                for dt in range(4):
                    psk = ps.tile([128, GROUP_DIMS], F32, tag="ps")
                    for i in range(8):
                        nc.tensor.matmul(psk[:, :SB],
                                         wk_t[:, i, dt * 128:(dt + 1) * 128],
                                         xp_t[:, i, :],
                                         start=(i == 0), stop=(i == 7))
                    nc.vector.tensor_scalar_add(
                        kT[:, dt, sb * SB:(sb + 1) * SB], psk[:, :SB],
                        bk_t[:, dt:dt + 1])
                for st in range(SB // 128):
                    psv = ps.tile([128, GROUP_DIMS], F32, tag="ps")
                    for i in range(8):
                        nc.tensor.matmul(psv[:],
                                         xp_t[:, i, st * 128:(st + 1) * 128],
                                         wv_t[:, i, :],
                                         start=(i == 0), stop=(i == 7))
                    pv = psv[:].rearrange("p (a c) -> p a c", a=4)
                    t = sb * 4 + st
                    nc.vector.tensor_copy(v_sb[:, t, :, 0:64], pv[:, :, 0:64])
                    nc.vector.tensor_copy(v_sb[:, t, :, 128:192], pv[:, :, 64:128])

            phase1_block(0)

            tri_t = cst.tile([128, 128], F32)
            ones_t = cst.tile([128, 64], BF16)
            nc.sync.dma_start(tri_t[:], tri_d[:])
            nc.sync.dma_start(ones_t[:], ones_d[:])
            wo_t = wts.tile([128, 4, D], BF16)
            bo_t = cst.tile([128, D], F32)
            for i in range(4):
                nc.sync.dma_start(wo_t[:, i, :], wo_d[i])
            nc.sync.dma_start(bo_t[:], bo_d[:])

            cci = dram.tile([S, D], F32)
            cco = dram.tile([S, D], F32)

            for qb in range(NQB):
                # ---- phase 2: attention for this query block, all head pairs
                for hp in range(4):
                    ps_oa = ps_acc.tile([128, QB], F32, tag="oa")
                    ps_ob = ps_acc.tile([128, QB], F32, tag="ob")
                    nkt = 4 * qb + 4
                    for kt in range(nkt):
                        d = kt - 4 * qb
                        n0 = max(0, 128 * d)
                        n1 = QB
                        qs0 = qb * QB + n0
                        qs1 = (qb + 1) * QB
                        s2 = ps_s.tile([128, 2, QB], F32, tag="s")
                        nc.tensor.matmul(s2[:, 0, n0:n1],
                                         kT[0:64, hp, kt * 128:(kt + 1) * 128],
                                         qT[0:64, hp, qs0:qs1],
                                         start=True, stop=True,
                                         tile_position=(0, 0))
                        nc.tensor.matmul(s2[:, 1, n0:n1],
                                         kT[64:128, hp, kt * 128:(kt + 1) * 128],
                                         qT[64:128, hp, qs0:qs1],
                                         start=True, stop=True,
                                         tile_position=(64, 0))
                        if d >= 0:
                            nc.vector.tensor_add(s2[:, 0, n0:n0 + 128],
                                                 s2[:, 0, n0:n0 + 128], tri_t[:])
                            nc.vector.tensor_add(s2[:, 1, n0:n0 + 128],
                                                 s2[:, 1, n0:n0 + 128], tri_t[:])
                        e2 = att.tile([128, 2, QB], BF16, tag="exp")
                        nc.scalar.activation(e2[:, :, n0:n1], s2[:, :, n0:n1],
                                             AFT.Exp)
                        first = kt == 0
                        last = kt == nkt - 1
                        # fused attn@v + rowsum: stationary [vA|1] / [1|vB]
                        nc.tensor.matmul(ps_oa[:, n0:n1],
                                         v_sb[:, kt, hp, 0:128],
                                         e2[:, 0, n0:n1], start=first, stop=last)
                        nc.tensor.matmul(ps_ob[:, n0:n1],
                                         v_sb[:, kt, hp, 128:256],
                                         e2[:, 1, n0:n1], start=first, stop=last)
                    # DVE may only touch PSUM with full-height base-0 APs
                    # (base-64 PSUM reads corrupt SBUF); stage to SBUF first
                    # and do all partition-shifted work there.
                    # head A: o rows 0:64, rowsum rows 64:128 -> shift down
                    sta = rcp.tile([128, QB], F32, tag="sta")
                    rta = rcp.tile([64, QB], F32, tag="rta")
                    rca = rcp.tile([64, QB], F32, tag="rca")
                    nc.vector.tensor_copy(sta[:], ps_oa[:])
                    nc.vector.tensor_copy(rta[:], sta[64:128, :])
                    nc.vector.reciprocal_approx_fast(rca[:], rta[:])
                    nc.vector.tensor_mul(coT[0:64, hp, qb * QB:(qb + 1) * QB],
                                         sta[0:64, :], rca[:])
                    # head B: same layout; normalize at base 0, DMA shifts
                    # the result up to coT rows 64:128 (DMA moves partitions
                    # freely; DVE shift-up copies corrupt memory).
                    stb = rcp.tile([128, QB], F32, tag="stb")
                    rtb = rcp.tile([64, QB], F32, tag="rtb")
                    rcb = rcp.tile([64, QB], F32, tag="rcb")
                    obn = rcp.tile([64, QB], BF16, tag="obn")
                    nc.vector.tensor_copy(stb[:], ps_ob[:])
                    nc.vector.tensor_copy(rtb[:], stb[64:128, :])
                    nc.vector.reciprocal_approx_fast(rcb[:], rtb[:])
                    nc.vector.tensor_mul(obn[:], stb[0:64, :], rcb[:])
                    nc.sync.dma_start(coT[64:128, hp, qb * QB:(qb + 1) * QB],
                                      obn[:])

                # ---- phase 1 for the next seq block (pipelined) ----
                if qb < NQB - 1:
                    phase1_block(qb + 1)

                # ---- phase 3 + chunked all-reduce for this 512-row chunk ----
                for st in range(4):
                    sq = 4 * qb + st
                    for ob in range(2):
                        pso = ps.tile([128, GROUP_DIMS], F32, tag="ps")
                        for ck in range(4):
                            nc.tensor.matmul(pso[:],
                                             coT[:, ck, sq * 128:(sq + 1) * 128],
                                             wo_t[:, ck, ob * 512:(ob + 1) * 512],
                                             start=(ck == 0), stop=(ck == 3))
                        ot = outp.tile([128, 512], F32, tag="out")
                        nc.vector.tensor_add(ot[:], pso[:],
                                             bo_t[:, ob * 512:(ob + 1) * 512])
                        nc.sync.dma_start(
                            cci[sq * 128:(sq + 1) * 128,
                                ob * 512:(ob + 1) * 512],
                            ot[:])
                nc.gpsimd.collective_compute(
                    "AllReduce",
                    mybir.AluOpType.add,
                    replica_groups=[[0, 1], [2, 3], [4, 5], [6, 7]],
                    ins=[cci[qb * QB:(qb + 1) * QB, :].opt()],
                    outs=[cco[qb * QB:(qb + 1) * QB, :].opt()],
                )
                for st in range(4):
                    sq = 4 * qb + st
                    nc.sync.dma_start(y_d[sq * 128:(sq + 1) * 128, :],
                                      cco[sq * 128:(sq + 1) * 128, :])

    nc.compile()
    return nc


def _get_program():
    global _PROG
    if _PROG is None:
        _PROG = _build_program()
    return _PROG


def kernel(x, pos_emb, Wq, bq, Wk, bk, Wv, bv, Wp, bp, Wo, bo):
    x = np.asarray(x, dtype=np.float32)
    pos_emb = np.asarray(pos_emb, dtype=np.float32)
    Wq, bq = np.asarray(Wq, np.float32), np.asarray(bq, np.float32)
    Wk, bk = np.asarray(Wk, np.float32), np.asarray(bk, np.float32)
    Wv, bv = np.asarray(Wv, np.float32), np.asarray(bv, np.float32)
    Wp, bp = np.asarray(Wp, np.float32), np.asarray(bp, np.float32)
    Wo, bo = np.asarray(Wo, np.float32), np.asarray(bo, np.float32)

    nc = _get_program()

    posT = np.ascontiguousarray(pos_emb.T)                      # [D, S]
    tri = np.where(np.arange(128)[:, None] <= np.arange(128)[None, :],
                   np.float32(0.0), np.float32(MASK_NEG)).astype(np.float32)
    ones64 = np.ones((128, 64), dtype=ml_dtypes.bfloat16)

    in_maps = []
    for c in range(N_CORES):
        b, g = divmod(c, 2)
        sl = slice(g * GROUP_DIMS, (g + 1) * GROUP_DIMS)
        xT = np.ascontiguousarray(x[b].T)                       # [D, S]
        xpos = np.concatenate([xT, posT], axis=0).reshape(16, 128, S)
        wqpT = np.concatenate([Wq[sl].T, Wp[sl].T], axis=0)     # [2D, 512]
        wkT = np.ascontiguousarray(Wk[sl].T)                    # [D, 512]
        wvT = np.ascontiguousarray(Wv[sl].T)
        woT = np.ascontiguousarray(Wo[:, sl].T)                 # [512, D]
        bqp = ((bq[sl] + bp[sl]) * 0.125).reshape(4, 128).T     # [128, 4]
        bk2 = bk[sl].reshape(4, 128).T
        bo_eff = bo * 0.5 + bv[sl] @ woT                        # [D]
        bo_bc = np.broadcast_to(bo_eff, (128, D))
        in_maps.append({
            "xpos": xpos.astype(ml_dtypes.bfloat16),
            "wqp": wqpT.reshape(16, 128, GROUP_DIMS).astype(ml_dtypes.bfloat16),
            "wk": wkT.reshape(8, 128, GROUP_DIMS).astype(ml_dtypes.bfloat16),
            "wv": wvT.reshape(8, 128, GROUP_DIMS).astype(ml_dtypes.bfloat16),
            "wo": woT.reshape(4, 128, D).astype(ml_dtypes.bfloat16),
            "bqp": np.ascontiguousarray(bqp, dtype=np.float32),
            "bk": np.ascontiguousarray(bk2, dtype=np.float32),
            "bo_bc": np.ascontiguousarray(bo_bc, dtype=np.float32),
            "tri": tri,
            "ones64": ones64,
        })

    global _last_in_maps
    _last_in_maps = in_maps

    res = run_bass_kernel_spmd(nc, in_maps, list(range(N_CORES)))
    out = np.stack([res.results[2 * b]["y"] for b in range(B)], axis=0)
    return out.astype(np.float32)
